# revision 43
# baseline (speedup 1.0000x reference)
"""Multi-head attention (B=4, d_model=512, N=2048, H=8) on 8 Trainium2 cores.

Sharding: core c handles batch b = c//2 and head-group hg = c%2 (4 heads).
Each core computes its heads' q/k/v projections, attention, and a partial
output merge (Wm restricted to its heads' channels).  The host sums the two
partials per batch and adds the folded bias (bm + Wm @ bv).

v2 layout (vs the earlier baseline):
  - x inputs and q/k/v projection weights ship as bf16 (halves input DMA;
    proj matmuls run bf16 at the same PE rate).
  - Scores run per head-PAIR with PE row tiling: head a occupies PE rows
    0-63, head b rows 64-127 (tile_position auto-derived from the k/q
    partition offsets), writing one shared [128, 2*NW] PSUM tile (separate
    banks per head) -> concurrent matmuls + ONE exp per pair.
  - n is processed in NW=512 windows (quarters); PSUM: s pair tile
    [128,1024] x2 bufs (4 banks) + x_ps a/b [65,512] (2 banks) + mg
    [128,512] x2 (2 banks) = 8 banks exactly.
  - Denominator: ones column appended to v^T (65th PV output row).
  - Minimal prelude: k_proj as xk slices arrive, q_proj(0), vt 0..3; all
    other projections / merges are interleaved into attention blocks.
  - Optional: a fraction of exp tiles computed on DVE via the Schraudolph
    int trick (bitcast(int32(A*s + B))) to offload the ACT bottleneck.
"""

import sys

for _p in ("/opt/trn_rl_repo",):
    if _p not in sys.path:
        sys.path.insert(0, _p)

from contextlib import ExitStack

import numpy as np

import concourse.bass as bass
import concourse.mybir as mybir
import concourse.tile as tile
from concourse import bacc
from concourse.bass_utils import run_bass_kernel_spmd

F32 = mybir.dt.float32
F32R = mybir.dt.float32r
BF16 = mybir.dt.bfloat16
I32 = mybir.dt.int32
EXP = mybir.ActivationFunctionType.Exp
ALU = mybir.AluOpType

B = 4
D = 512  # d_model
N = 2048
H = 8
HD = 64  # head dim
HPC = 4  # heads per core
C = HPC * HD  # 256 local channels per core
KT = D // 128  # contraction tiles for projections
CT = C // 128  # local channel tiles
MT = N // 128  # tiles of the m (key position) axis
NW = 512  # n window (quarter)
NQ = N // NW
NCORES = 8

# Schraudolph exp: exp(s/8) ~= bitcast_f32(int32(s * SCHRA_A + SCHRA_B)).
# B calibrated numerically for the N(0,1) score/8 distribution.
SCHRA_A = (2.0**23 / np.log(2.0)) / 8.0
SCHRA_B = 1064866750.0
# m-tiles whose exp runs on DVE (approx) instead of ACT (exact); applied
# from global step OFFLOAD_START onward (prelude keeps DVE free early on)
OFFLOAD_MTS = ()
OFFLOAD_START = 16


def _mm(nc, out, lhsT, rhs, **kw):
    nc.tensor.matmul(out, lhsT, rhs, skip_group_check=True, **kw)


def build_program(reps=1, offload_mts=OFFLOAD_MTS, debug=False, batched_dma=True):
    NH_ONES = HPC * (HD + 1)

    nc = bacc.Bacc(
        "TRN2",
        target_bir_lowering=False,
        debug=False,
        enable_asserts=False,
        num_devices=NCORES,
    )

    xq_d = nc.declare_dram_parameter("xq", [D, N], BF16, isOutput=False).ap()
    xk_d = nc.declare_dram_parameter("xk", [D, N], BF16, isOutput=False).ap()
    xv_d = nc.declare_dram_parameter("xv", [D, N], BF16, isOutput=False).ap()
    wq_d = nc.declare_dram_parameter("wqT", [D, C], BF16, isOutput=False).ap()
    wk_d = nc.declare_dram_parameter("wkT", [D, C], BF16, isOutput=False).ap()
    wv_d = nc.declare_dram_parameter("wvT", [D, C], BF16, isOutput=False).ap()
    wm_d = nc.declare_dram_parameter("wmT", [C, D], F32R, isOutput=False).ap()
    bq_d = nc.declare_dram_parameter("bq2", [128, CT], F32, isOutput=False).ap()
    on_d = nc.declare_dram_parameter("onesc", [128, HPC], F32R, isOutput=False).ap()
    out_d = nc.declare_dram_parameter("out", [D, N], F32, isOutput=True).ap()
    if debug:
        dbg_q = nc.declare_dram_parameter("dbg_q", [128, N], F32, isOutput=True).ap()
        dbg_k = nc.declare_dram_parameter("dbg_k", [128, N], F32, isOutput=True).ap()
        dbg_vt = nc.declare_dram_parameter(
            "dbg_vt", [128, HPC * (HD + 1)], F32, isOutput=True
        ).ap()
        dbg_pr = nc.declare_dram_parameter("dbg_pr", [128, 2 * NW], F32, isOutput=True).ap()
        dbg_x = nc.declare_dram_parameter("dbg_x", [128, N], F32, isOutput=True).ap()
        dbg_xf = nc.declare_dram_parameter("dbg_xf", [HD + 1, NW], F32, isOutput=True).ap()

    with tile.TileContext(nc) as tc, ExitStack() as ctx:
        big = ctx.enter_context(tc.tile_pool(name="big", bufs=1))
        wp = ctx.enter_context(tc.tile_pool(name="wp", bufs=1))
        pk = ctx.enter_context(tc.tile_pool(name="pk", bufs=1))
        sm = ctx.enter_context(tc.tile_pool(name="sm", bufs=2))
        pp = ctx.enter_context(tc.tile_pool(name="pp", bufs=1, space="PSUM"))

        def emit_body(rep):
            if rep == 0:
                # ---- ACT exp-table preload --------------------------------
                warm = wp.tile([1, 16], F32, tag="warm", name="warm")
                nc.vector.memset(warm, 0.0)
                nc.scalar.activation(warm[0:1, 8:16], warm[0:1, 0:8], EXP, scale=1.0)

            # ---- SBUF tiles (kt-blocks packed in one tile per tensor so a
            # single DMA with a 3D access pattern loads everything; HWDGE
            # descriptor-gen is ~0.6us per DMA and dominates small loads) ----
            wk_all = wp.tile([128, KT * C], BF16, tag="wk", name="wk")
            wq_all = wp.tile([128, KT * C], BF16, tag="wq", name="wq")
            wv_all = wp.tile([128, KT * C], BF16, tag="wv", name="wv")
            wm_all = wp.tile([128, CT * D], F32R, tag="wm", name="wm")
            wk_sb = [wk_all[:, kt * C : (kt + 1) * C] for kt in range(KT)]
            wq_sb = [wq_all[:, kt * C : (kt + 1) * C] for kt in range(KT)]
            wv_sb = [wv_all[:, kt * C : (kt + 1) * C] for kt in range(KT)]
            wm_sb = [wm_all[:, ct * D : (ct + 1) * D] for ct in range(CT)]
            bq_sb = wp.tile([128, CT], F32, tag="bq", name="bq")
            on_sb = wp.tile([128, HPC], F32R, tag="onesc", name="onesc")

            xq_all = big.tile([128, KT * N], BF16, tag="xq", name="xq")
            xk_all = big.tile([128, KT * N], BF16, tag="xk", name="xk")
            xv_all = big.tile([128, KT * N], BF16, tag="xv", name="xv")
            xq_sb = [xq_all[:, kt * N : (kt + 1) * N] for kt in range(KT)]
            xk_sb = [xk_all[:, kt * N : (kt + 1) * N] for kt in range(KT)]
            xv_sb = [xv_all[:, kt * N : (kt + 1) * N] for kt in range(KT)]

            q_sb, k_sb, x_sb = [], [], []
            for ct in range(CT):
                q_sb.append(pk.tile([128, N], F32R, tag=f"q{ct}", name=f"q{ct}"))
                k_sb.append(pk.tile([128, N], F32R, tag=f"k{ct}", name=f"k{ct}"))
                x_sb.append(pk.tile([128, N], F32R, tag=f"x{ct}", name=f"x{ct}"))
            vt_sb = []
            for mt in range(MT):
                vt_sb.append(
                    pk.tile([128, NH_ONES], F32R, tag=f"vt{mt}", name=f"vt{mt}")
                )

            # ---- DMA loads: one 3D-AP transfer per (tensor, col range) -----
            def load3(dst_all, src_d, cols, blocks=KT):
                if batched_dma:
                    nc.sync.dma_start(
                        dst_all.rearrange("p (k n) -> p k n", k=blocks)[:, :, cols],
                        src_d.rearrange("(k p) n -> p k n", p=128)[:, :, cols],
                    )
                    return
                nfull = dst_all.shape[1] // blocks
                for kt in range(blocks):
                    nc.sync.dma_start(
                        dst_all[:, kt * nfull : (kt + 1) * nfull][:, cols],
                        src_d[kt * 128 : (kt + 1) * 128, cols],
                    )

            load3(wk_all, wk_d, slice(0, C))
            load3(xk_all, xk_d, slice(0, NW))
            load3(wq_all, wq_d, slice(0, C))
            nc.sync.dma_start(bq_sb, bq_d)
            nc.sync.dma_start(on_sb, on_d)
            load3(xq_all, xq_d, slice(0, NW))
            load3(wv_all, wv_d, slice(0, C))
            load3(xv_all, xv_d, slice(0, NW))
            load3(xk_all, xk_d, slice(NW, 2 * NW))
            load3(xk_all, xk_d, slice(2 * NW, N))
            load3(xv_all, xv_d, slice(NW, 2 * NW))
            load3(xv_all, xv_d, slice(2 * NW, N))
            load3(xq_all, xq_d, slice(NW, N))
            load3(wm_all, wm_d, slice(0, D), CT)

            # ---- projection emitters --------------------------------------
            # ptag="s": prelude work borrows the attention score buffers,
            # whose previous-rep WAR (the last exps) clears far earlier than
            # "mg" (held by the previous rep's tail merges) -> back-to-back
            # reps overlap their prelude with the prior rep's tail.
            def proj_ps(ptag, name):
                if ptag == "s":
                    t = pp.tile([128, 2 * NW], F32, tag="s", bufs=2, name=name)
                    return t[:, 0:NW]
                return pp.tile([128, NW], F32, tag="mg", bufs=2, name=name)

            def k_proj(j, on_act, cts=range(CT), ptag="mg"):
                js = slice(j * NW, (j + 1) * NW)
                for ct in cts:
                    ps = proj_ps(ptag, f"kps{ct}_{j}")
                    for kt in range(KT):
                        _mm(
                            nc, ps,
                            wk_sb[kt][:, ct * 128 : (ct + 1) * 128],
                            xk_sb[kt][:, js],
                            start=(kt == 0), stop=(kt == KT - 1),
                        )
                    if on_act:
                        nc.scalar.copy(k_sb[ct][:, js], ps)
                    else:
                        nc.vector.tensor_copy(k_sb[ct][:, js], ps)

            def q_proj(j, on_act, cts=range(CT), ptag="mg"):
                js = slice(j * NW, (j + 1) * NW)
                for ct in cts:
                    ps = proj_ps(ptag, f"qps{ct}_{j}")
                    for kt in range(KT):
                        _mm(
                            nc, ps,
                            wq_sb[kt][:, ct * 128 : (ct + 1) * 128],
                            xq_sb[kt][:, js],
                            start=(kt == 0), stop=(kt == KT - 1),
                        )
                    if on_act:
                        nc.scalar.add(q_sb[ct][:, js], ps, bq_sb[:, ct : ct + 1])
                    else:
                        nc.vector.tensor_scalar_add(
                            q_sb[ct][:, js], ps, bq_sb[:, ct : ct + 1]
                        )

            def vt_proj(mt, ptag="mg"):
                t3 = vt_sb[mt].rearrange("p (h x) -> p h x", h=HPC)
                ps = proj_ps(ptag, f"vps{mt}")
                for kt in range(KT):
                    _mm(
                        nc, ps[:, 0:C],
                        xv_sb[kt][:, mt * 128 : (mt + 1) * 128],
                        wv_sb[kt],
                        start=(kt == 0), stop=(kt == KT - 1),
                    )
                nc.vector.tensor_copy(
                    t3[:, :, 0:HD], ps[:, 0:C].rearrange("p (h d) -> p h d", h=HPC)
                )
                nc.vector.tensor_copy(
                    t3[:, :, HD : HD + 1], on_sb.rearrange("p (h o) -> p h o", o=1)
                )

            # ---- merge: out[ot*128:, qn window] from x_sb ------------------
            def merge_unit(ot, qn, on_act=False):
                gjs = slice(qn * NW, (qn + 1) * NW)
                ps = pp.tile([128, NW], F32, tag="mg", bufs=2, name=f"ops{ot}_{qn}")
                for ct in range(CT):
                    _mm(
                        nc, ps,
                        wm_sb[ct][:, ot * 128 : (ot + 1) * 128],
                        x_sb[ct][:, gjs],
                        start=(ct == 0), stop=(ct == CT - 1),
                    )
                ob = sm.tile([128, NW], F32, tag=f"ob{ot % 2}", bufs=2, name=f"ob{ot}_{qn}")
                if on_act:
                    nc.scalar.copy(ob, ps)
                else:
                    nc.vector.tensor_copy(ob, ps)
                nc.sync.dma_start(out_d[ot * 128 : (ot + 1) * 128, gjs], ob)

            # ---- attention: flat pipeline over (block, mt) steps -----------
            # block order: (hp0,q0) (hp1,q0) (hp0,q1) (hp1,q1) ...
            border = [(hp, qn) for qn in range(NQ) for hp in range(2)]
            steps = [(hp, qn, mt) for (hp, qn) in border for mt in range(MT)]
            NS = len(steps)
            s_tiles = {}
            x_ps = {}  # (hp, qn) -> {a: tile, b: tile}

            def qk(i):
                hp, qn, mt = steps[i]
                gjs = slice(qn * NW, (qn + 1) * NW)
                ms = slice(mt * 128, (mt + 1) * 128)
                st = pp.tile([128, 2 * NW], F32, tag="s", bufs=2, name=f"s{i}")
                s_tiles[i] = st
                # head a: PE rows 0-63; head b: rows 64-127 (row tiling)
                _mm(
                    nc, st[:, 0:NW],
                    k_sb[hp][0:64, ms], q_sb[hp][0:64, gjs],
                    start=True, stop=True, tile_position=(0, 0),
                )
                _mm(
                    nc, st[:, NW : 2 * NW],
                    k_sb[hp][64:128, ms], q_sb[hp][64:128, gjs],
                    start=True, stop=True, tile_position=(64, 0),
                )

            def normalize(hp, qn, last=False):
                # One fast PSUM->SBUF copy frees x_ps for the next block's
                # first PV; the recip/broadcast/mul chain then runs off the
                # PE-critical path from the SBUF staging copy.  The final
                # block skips the staging copy (nothing waits on x_ps).
                gjs = slice(qn * NW, (qn + 1) * NW)
                for i, hnm in enumerate("ab"):
                    xp = x_ps[(hp, qn)][hnm]
                    if not last:
                        xf = sm.tile(
                            [HD + 1, NW], F32, tag=f"xf{hnm}", bufs=2,
                            name=f"xf{hnm}{hp}{qn}",
                        )
                        nc.vector.tensor_copy(xf, xp)
                        if debug and hp == 0 and qn == 0 and hnm == "a":
                            nc.sync.dma_start(dbg_xf, xf)
                    else:
                        xf = xp
                    den = sm.tile([1, NW], F32, tag="den", name=f"dn{hnm}{hp}{qn}")
                    nc.vector.tensor_copy(den, xp[HD : HD + 1, :])
                    recip = sm.tile([1, NW], F32, tag="rec", name=f"rc{hnm}{hp}{qn}")
                    nc.vector.reciprocal_approx_fast(out=recip, in_=den)
                    bc = sm.tile([64, NW], F32, tag="bc", bufs=2, name=f"bc{hnm}{hp}{qn}")
                    nc.gpsimd.partition_broadcast(bc, recip)
                    rows = slice(i * 64, i * 64 + 64)
                    nc.vector.tensor_mul(x_sb[hp][rows, gjs], xf[0:HD, :], bc)

            # ---- prelude: only the minimum before the first qk pair; the
            # rest is emitted right after qk(0)/qk(1) and overlaps the first
            # attention steps.  Everything runs on the "s" psum tag with DVE
            # copies so a chained rep's prelude overlaps the previous rep's
            # tail (ACT queue stays exp-only, "mg" stays with the old tail).

            # ---- hooks keyed by global step --------------------------------
            from collections import defaultdict
            from functools import partial

            hooks = defaultdict(list)
            # vt_proj 7..15 (PV(mt) is emitted at step mt+2)
            for mt in range(7, MT):
                hooks[mt - 3].append(partial(vt_proj, mt))
            # q_proj(qn+1): ct0 before block 2(qn+1) (step 32(qn+1)-2),
            # ct1 before block 2(qn+1)+1
            for qn in range(NQ - 1):
                hooks[32 * qn + 13].append(partial(q_proj, qn + 1, False, [0]))
                hooks[32 * qn + 21].append(partial(q_proj, qn + 1, False, [1]))
            # merge(qn) interleaved into the blocks of window qn+1
            for qn in range(NQ - 1):
                for ot in range(KT):
                    blk = 2 * (qn + 1) + (ot // 2)
                    hooks[blk * MT + 5 + 6 * (ot % 2)].append(partial(merge_unit, ot, qn))

            pr_tiles = {}

            def is_off(i):
                return steps[i][2] in offload_mts and i >= OFFLOAD_START

            # PV emission is deferred: exact tiles 2 steps after their exp
            # (3 for mt==0 to decouple x_ps reuse from the previous block's
            # staging copy), offloaded tiles 6 steps (DVE latency headroom).
            # The block's last-emitted PV carries stop=True.
            def pv_due(i):
                mt = steps[i][2]
                if i >= NS - MT:  # last block: nothing left to overlap with
                    return i + (4 if is_off(i) else 2 if mt == 0 else 1)
                return i + (6 if is_off(i) else 3 if mt == 0 else 2)

            last_pv = {}  # block index -> step that must carry stop=True
            for i in range(NS):
                last_pv[i // MT] = max(
                    last_pv.get(i // MT, (0, 0)), (pv_due(i), i)
                )
            last_pv = {b: j for b, (_, j) in last_pv.items()}

            def pv(i):
                hp, qn, mt = steps[i]
                if mt == 0:
                    x_ps[(hp, qn)] = {
                        hnm: pp.tile(
                            [HD + 1, NW], F32, tag=f"x{hnm}", bufs=1,
                            name=f"xp{hnm}{hp}{qn}",
                        )
                        for hnm in "ab"
                    }
                pr = pr_tiles.pop(i)
                stop = last_pv[i // MT] == i
                for j, hnm in enumerate("ab"):
                    _mm(
                        nc, x_ps[(hp, qn)][hnm],
                        vt_sb[mt][:, (2 * hp + j) * (HD + 1) : (2 * hp + j + 1) * (HD + 1)],
                        pr[:, j * NW : (j + 1) * NW],
                        start=(mt == 0), stop=stop,
                    )
                if stop:
                    normalize(hp, qn, last=(i // MT == len(border) - 1))

            it_tiles = {}

            def op1(i):
                # Schraudolph exp on DVE, hoisted one step early: builds int
                # bits from PSUM scores as soon as qk(i) lands, so the s-buf
                # frees without gating the qk chain on DVE latency.
                it = sm.tile(
                    [128, 2 * NW], I32, tag=f"it{i % 2}", bufs=1, name=f"it{i}"
                )
                nc.vector.tensor_scalar(
                    it, s_tiles.pop(i), SCHRA_A, SCHRA_B, op0=ALU.mult, op1=ALU.add
                )
                it_tiles[i] = it

            import heapq

            pv_q = []  # (due, step)
            noff = 0

            k_proj(0, on_act=False, ptag="s")
            q_proj(0, on_act=False, ptag="s")
            qk(0)
            qk(1)
            for g in range(1, NQ):
                k_proj(g, on_act=False, ptag="s")
            for mt in range(7):
                vt_proj(mt, ptag="s")
            if is_off(0):
                op1(0)
            for i in range(NS):
                hp, qn, mt = steps[i]
                if i + 1 < NS and is_off(i + 1):
                    op1(i + 1)
                if is_off(i):
                    pr = sm.tile(
                        [128, 2 * NW], F32R, tag=f"po{noff % 3}", bufs=1, name=f"pr{i}"
                    )
                    noff += 1
                    nc.vector.tensor_copy(pr, it_tiles.pop(i).bitcast(F32))
                else:
                    pr = sm.tile(
                        [128, 2 * NW], F32R, tag=f"pr{i % 4}", bufs=1, name=f"pr{i}"
                    )
                    nc.scalar.activation(pr, s_tiles.pop(i), EXP, scale=0.125)
                pr_tiles[i] = pr
                if debug and i == 0:
                    nc.sync.dma_start(dbg_pr, pr.bitcast(F32))
                if i + 2 < NS:
                    qk(i + 2)
                heapq.heappush(pv_q, (pv_due(i), i))
                while pv_q and pv_q[0][0] <= i:
                    pv(heapq.heappop(pv_q)[1])
                for fn in hooks.get(i, ()):
                    fn()
            while pv_q:
                pv(heapq.heappop(pv_q)[1])
            # tail: last window's merges
            for ot in range(KT):
                merge_unit(ot, NQ - 1, on_act=(ot % 2 == 0))
            if debug:
                nc.sync.dma_start(dbg_q, q_sb[0].bitcast(F32))
                nc.sync.dma_start(dbg_k, k_sb[0].bitcast(F32))
                nc.sync.dma_start(dbg_vt, vt_sb[0].bitcast(F32))
                nc.sync.dma_start(dbg_x, x_sb[0].bitcast(F32))

        for rep in range(reps):
            emit_body(rep)

    nc.compile()
    return nc


def make_in_maps(query, key, value, Wq, bq, Wk, Wv, Wm, n_cores=NCORES):
    import ml_dtypes

    bf16 = ml_dtypes.bfloat16
    query = np.asarray(query, np.float32)
    key = np.asarray(key, np.float32)
    value = np.asarray(value, np.float32)
    Wq = np.asarray(Wq, np.float32)
    bq = np.asarray(bq, np.float32)
    Wk = np.asarray(Wk, np.float32)
    Wv = np.asarray(Wv, np.float32)
    Wm = np.asarray(Wm, np.float32)
    in_maps = []
    for c in range(n_cores):
        b, hg = c // 2, c % 2
        heads = [hg * HPC + i for i in range(HPC)]
        mych = np.array([d * H + h for h in heads for d in range(HD)])
        in_maps.append(
            {
                "xq": np.ascontiguousarray(query[b].astype(bf16)),
                "xk": np.ascontiguousarray(key[b].astype(bf16)),
                "xv": np.ascontiguousarray(value[b].astype(bf16)),
                "wqT": np.ascontiguousarray(Wq[mych].T.astype(bf16)),
                "wkT": np.ascontiguousarray(Wk[mych].T.astype(bf16)),
                "wvT": np.ascontiguousarray(Wv[mych].T.astype(bf16)),
                "wmT": np.ascontiguousarray(Wm[:, mych].T),
                "bq2": np.ascontiguousarray(bq[mych].reshape(CT, 128).T),
                "onesc": np.ones((128, HPC), np.float32),
            }
        )
    return in_maps


def prep_inputs(inputs):
    """dev_hwtime.py hook: full-input dict -> per-core in_maps."""
    return make_in_maps(
        inputs["query"], inputs["key"], inputs["value"], inputs["Wq"],
        inputs["bq"], inputs["Wk"], inputs["Wv"], inputs["Wm"],
    )


_PROG = {}


def _get_program():
    if "p" not in _PROG:
        _PROG["p"] = build_program()
    return _PROG["p"]


def kernel(query, key, value, Wq, bq, Wk, bk, Wv, bv, Wm, bm):
    nc = _get_program()
    in_maps = make_in_maps(query, key, value, Wq, bq, Wk, Wv, Wm)
    res = run_bass_kernel_spmd(nc, in_maps, list(range(NCORES))).results
    bm_eff = (np.asarray(Wm, np.float64) @ np.asarray(bv, np.float64)).astype(
        np.float32
    ) + np.asarray(bm, np.float32)
    out = np.empty((B, D, N), np.float32)
    for b in range(B):
        out[b] = res[2 * b]["out"] + res[2 * b + 1]["out"] + bm_eff[:, None]
    return out


# revision 47
# speedup vs baseline: 1.0530x; 1.0530x over previous
"""Multi-head attention (B=4, d_model=512, N=2048, H=8) on 8 Trainium2 cores.

Sharding: core c handles batch b = c//2 and head-group hg = c%2 (4 heads).
Each core computes its heads' q/k/v projections, attention, and a partial
output merge (Wm restricted to its heads' channels).  The host sums the two
partials per batch and adds the folded bias (bm + Wm @ bv).

v2 layout (vs the earlier baseline):
  - x inputs and q/k/v projection weights ship as bf16 (halves input DMA;
    proj matmuls run bf16 at the same PE rate).
  - Scores run per head-PAIR with PE row tiling: head a occupies PE rows
    0-63, head b rows 64-127 (tile_position auto-derived from the k/q
    partition offsets), writing one shared [128, 2*NW] PSUM tile (separate
    banks per head) -> concurrent matmuls + ONE exp per pair.
  - n is processed in NW=512 windows (quarters); PSUM: s pair tile
    [128,1024] x2 bufs (4 banks) + x_ps a/b [65,512] (2 banks) + mg
    [128,512] x2 (2 banks) = 8 banks exactly.
  - Denominator: ones column appended to v^T (65th PV output row).
  - Minimal prelude: k_proj as xk slices arrive, q_proj(0), vt 0..3; all
    other projections / merges are interleaved into attention blocks.
  - Optional: a fraction of exp tiles computed on DVE via the Schraudolph
    int trick (bitcast(int32(A*s + B))) to offload the ACT bottleneck.
"""

import sys

for _p in ("/opt/trn_rl_repo",):
    if _p not in sys.path:
        sys.path.insert(0, _p)

from contextlib import ExitStack

import numpy as np

import concourse.bass as bass
import concourse.mybir as mybir
import concourse.tile as tile
from concourse import bacc
from concourse.bass_utils import run_bass_kernel_spmd

F32 = mybir.dt.float32
F32R = mybir.dt.float32r
BF16 = mybir.dt.bfloat16
I32 = mybir.dt.int32
EXP = mybir.ActivationFunctionType.Exp
ALU = mybir.AluOpType

B = 4
D = 512  # d_model
N = 2048
H = 8
HD = 64  # head dim
HPC = 4  # heads per core
C = HPC * HD  # 256 local channels per core
KT = D // 128  # contraction tiles for projections
CT = C // 128  # local channel tiles
MT = N // 128  # tiles of the m (key position) axis
NW = 512  # n window (quarter)
NQ = N // NW
NCORES = 8

# Schraudolph exp: exp(s/8) ~= bitcast_f32(int32(s * SCHRA_A + SCHRA_B)).
# B calibrated numerically for the N(0,1) score/8 distribution.
SCHRA_A = (2.0**23 / np.log(2.0)) / 8.0
SCHRA_B = 1064866750.0
# m-tiles whose exp runs on DVE (approx) instead of ACT (exact); applied
# from global step OFFLOAD_START onward (prelude keeps DVE free early on)
OFFLOAD_MTS = ()
OFFLOAD_START = 16


def _mm(nc, out, lhsT, rhs, **kw):
    nc.tensor.matmul(out, lhsT, rhs, skip_group_check=True, **kw)


def build_program(reps=1, offload_mts=OFFLOAD_MTS, debug=False, batched_dma=True):
    NH_ONES = HPC * (HD + 1)

    nc = bacc.Bacc(
        "TRN2",
        target_bir_lowering=False,
        debug=False,
        enable_asserts=False,
        num_devices=NCORES,
    )

    xq_d = nc.declare_dram_parameter("xq", [D, N], BF16, isOutput=False).ap()
    xk_d = nc.declare_dram_parameter("xk", [D, N], BF16, isOutput=False).ap()
    xv_d = nc.declare_dram_parameter("xv", [D, N], BF16, isOutput=False).ap()
    wq_d = nc.declare_dram_parameter("wqT", [D, C], BF16, isOutput=False).ap()
    wk_d = nc.declare_dram_parameter("wkT", [D, C], BF16, isOutput=False).ap()
    wv_d = nc.declare_dram_parameter("wvT", [D, C], BF16, isOutput=False).ap()
    wm_d = nc.declare_dram_parameter("wmT", [C, D], F32R, isOutput=False).ap()
    bq_d = nc.declare_dram_parameter("bq2", [128, CT], F32, isOutput=False).ap()
    on_d = nc.declare_dram_parameter("onesc", [128, HPC], F32R, isOutput=False).ap()
    out_d = nc.declare_dram_parameter("out", [D, N], F32, isOutput=True).ap()
    if debug:
        dbg_q = nc.declare_dram_parameter("dbg_q", [128, N], F32, isOutput=True).ap()
        dbg_k = nc.declare_dram_parameter("dbg_k", [128, N], F32, isOutput=True).ap()
        dbg_vt = nc.declare_dram_parameter(
            "dbg_vt", [128, HPC * (HD + 1)], F32, isOutput=True
        ).ap()
        dbg_pr = nc.declare_dram_parameter("dbg_pr", [128, 2 * NW], F32, isOutput=True).ap()
        dbg_x = nc.declare_dram_parameter("dbg_x", [128, N], F32, isOutput=True).ap()
        dbg_xf = nc.declare_dram_parameter("dbg_xf", [HD + 1, NW], F32, isOutput=True).ap()

    with tile.TileContext(nc) as tc, ExitStack() as ctx:
        big = ctx.enter_context(tc.tile_pool(name="big", bufs=1))
        wp = ctx.enter_context(tc.tile_pool(name="wp", bufs=1))
        pk = ctx.enter_context(tc.tile_pool(name="pk", bufs=1))
        sm = ctx.enter_context(tc.tile_pool(name="sm", bufs=2))
        pp = ctx.enter_context(tc.tile_pool(name="pp", bufs=1, space="PSUM"))

        def emit_body(rep):
            if rep == 0:
                # ---- ACT exp-table preload --------------------------------
                warm = wp.tile([1, 16], F32, tag="warm", name="warm")
                nc.vector.memset(warm, 0.0)
                nc.scalar.activation(warm[0:1, 8:16], warm[0:1, 0:8], EXP, scale=1.0)

            # ---- SBUF tiles (kt-blocks packed in one tile per tensor so a
            # single DMA with a 3D access pattern loads everything; HWDGE
            # descriptor-gen is ~0.6us per DMA and dominates small loads) ----
            wk_all = wp.tile([128, KT * C], BF16, tag="wk", name="wk")
            wq_all = wp.tile([128, KT * C], BF16, tag="wq", name="wq")
            wv_all = wp.tile([128, KT * C], BF16, tag="wv", name="wv")
            wm_all = wp.tile([128, CT * D], F32R, tag="wm", name="wm")
            wk_sb = [wk_all[:, kt * C : (kt + 1) * C] for kt in range(KT)]
            wq_sb = [wq_all[:, kt * C : (kt + 1) * C] for kt in range(KT)]
            wv_sb = [wv_all[:, kt * C : (kt + 1) * C] for kt in range(KT)]
            wm_sb = [wm_all[:, ct * D : (ct + 1) * D] for ct in range(CT)]
            bq_sb = wp.tile([128, CT], F32, tag="bq", name="bq")
            on_sb = wp.tile([128, HPC], F32R, tag="onesc", name="onesc")

            xq_all = big.tile([128, KT * N], BF16, tag="xq", name="xq")
            xk_all = big.tile([128, KT * N], BF16, tag="xk", name="xk")
            xv_all = big.tile([128, KT * N], BF16, tag="xv", name="xv")
            xq_sb = [xq_all[:, kt * N : (kt + 1) * N] for kt in range(KT)]
            xk_sb = [xk_all[:, kt * N : (kt + 1) * N] for kt in range(KT)]
            xv_sb = [xv_all[:, kt * N : (kt + 1) * N] for kt in range(KT)]

            q_sb, k_sb, x_sb = [], [], []
            for ct in range(CT):
                q_sb.append(pk.tile([128, N], F32R, tag=f"q{ct}", name=f"q{ct}"))
                k_sb.append(pk.tile([128, N], F32R, tag=f"k{ct}", name=f"k{ct}"))
                x_sb.append(pk.tile([128, N], F32R, tag=f"x{ct}", name=f"x{ct}"))
            vt_sb = []
            for mt in range(MT):
                vt_sb.append(
                    pk.tile([128, NH_ONES], F32R, tag=f"vt{mt}", name=f"vt{mt}")
                )

            # ---- DMA loads: one 3D-AP transfer per (tensor, col range) -----
            def load3(dst_all, src_d, cols, blocks=KT):
                if batched_dma:
                    nc.sync.dma_start(
                        dst_all.rearrange("p (k n) -> p k n", k=blocks)[:, :, cols],
                        src_d.rearrange("(k p) n -> p k n", p=128)[:, :, cols],
                    )
                    return
                nfull = dst_all.shape[1] // blocks
                for kt in range(blocks):
                    nc.sync.dma_start(
                        dst_all[:, kt * nfull : (kt + 1) * nfull][:, cols],
                        src_d[kt * 128 : (kt + 1) * 128, cols],
                    )

            load3(wk_all, wk_d, slice(0, C))
            load3(xk_all, xk_d, slice(0, NW))
            load3(wq_all, wq_d, slice(0, C))
            nc.sync.dma_start(bq_sb, bq_d)
            nc.sync.dma_start(on_sb, on_d)
            load3(xq_all, xq_d, slice(0, NW))
            load3(wv_all, wv_d, slice(0, C))
            load3(xv_all, xv_d, slice(0, NW))
            load3(xk_all, xk_d, slice(NW, 2 * NW))
            load3(xk_all, xk_d, slice(2 * NW, N))
            load3(xv_all, xv_d, slice(NW, 2 * NW))
            load3(xv_all, xv_d, slice(2 * NW, N))
            load3(xq_all, xq_d, slice(NW, N))
            load3(wm_all, wm_d, slice(0, D), CT)

            # ---- projection emitters --------------------------------------
            # ptag="s": prelude work borrows the attention score buffers,
            # whose previous-rep WAR (the last exps) clears far earlier than
            # "mg" (held by the previous rep's tail merges) -> back-to-back
            # reps overlap their prelude with the prior rep's tail.
            def proj_ps(ptag, name):
                if ptag == "s":
                    t = pp.tile([128, 2 * NW], F32, tag="s", bufs=2, name=name)
                    return t[:, 0:NW]
                return pp.tile([128, NW], F32, tag="mg", bufs=2, name=name)

            def k_proj(j, on_act, cts=range(CT), ptag="mg"):
                js = slice(j * NW, (j + 1) * NW)
                for ct in cts:
                    ps = proj_ps(ptag, f"kps{ct}_{j}")
                    for kt in range(KT):
                        _mm(
                            nc, ps,
                            wk_sb[kt][:, ct * 128 : (ct + 1) * 128],
                            xk_sb[kt][:, js],
                            start=(kt == 0), stop=(kt == KT - 1),
                        )
                    if on_act:
                        nc.scalar.copy(k_sb[ct][:, js], ps)
                    else:
                        nc.vector.tensor_copy(k_sb[ct][:, js], ps)

            def q_proj(j, on_act, cts=range(CT), ptag="mg"):
                js = slice(j * NW, (j + 1) * NW)
                for ct in cts:
                    ps = proj_ps(ptag, f"qps{ct}_{j}")
                    for kt in range(KT):
                        _mm(
                            nc, ps,
                            wq_sb[kt][:, ct * 128 : (ct + 1) * 128],
                            xq_sb[kt][:, js],
                            start=(kt == 0), stop=(kt == KT - 1),
                        )
                    if on_act:
                        nc.scalar.add(q_sb[ct][:, js], ps, bq_sb[:, ct : ct + 1])
                    else:
                        nc.vector.tensor_scalar_add(
                            q_sb[ct][:, js], ps, bq_sb[:, ct : ct + 1]
                        )

            def vt_proj(mt, ptag="mg"):
                t3 = vt_sb[mt].rearrange("p (h x) -> p h x", h=HPC)
                ps = proj_ps(ptag, f"vps{mt}")
                for kt in range(KT):
                    _mm(
                        nc, ps[:, 0:C],
                        xv_sb[kt][:, mt * 128 : (mt + 1) * 128],
                        wv_sb[kt],
                        start=(kt == 0), stop=(kt == KT - 1),
                    )
                nc.vector.tensor_copy(
                    t3[:, :, 0:HD], ps[:, 0:C].rearrange("p (h d) -> p h d", h=HPC)
                )
                nc.vector.tensor_copy(
                    t3[:, :, HD : HD + 1], on_sb.rearrange("p (h o) -> p h o", o=1)
                )

            # ---- merge: out[ot*128:, qn window] from x_sb ------------------
            def merge_unit(ot, qn, on_act=False):
                gjs = slice(qn * NW, (qn + 1) * NW)
                ps = pp.tile([128, NW], F32, tag="mg", bufs=2, name=f"ops{ot}_{qn}")
                for ct in range(CT):
                    _mm(
                        nc, ps,
                        wm_sb[ct][:, ot * 128 : (ot + 1) * 128],
                        x_sb[ct][:, gjs],
                        start=(ct == 0), stop=(ct == CT - 1),
                    )
                ob = sm.tile([128, NW], F32, tag=f"ob{ot % 2}", bufs=2, name=f"ob{ot}_{qn}")
                if on_act:
                    nc.scalar.copy(ob, ps)
                else:
                    nc.vector.tensor_copy(ob, ps)
                nc.sync.dma_start(out_d[ot * 128 : (ot + 1) * 128, gjs], ob)

            # ---- attention: flat pipeline over (block, mt) steps -----------
            # block order: (hp0,q0) (hp1,q0) (hp0,q1) (hp1,q1) ...
            border = [(hp, qn) for qn in range(NQ) for hp in range(2)]
            steps = [(hp, qn, mt) for (hp, qn) in border for mt in range(MT)]
            NS = len(steps)
            s_tiles = {}
            x_ps = {}  # (hp, qn) -> {a: tile, b: tile}

            def qk(i):
                hp, qn, mt = steps[i]
                gjs = slice(qn * NW, (qn + 1) * NW)
                ms = slice(mt * 128, (mt + 1) * 128)
                st = pp.tile([128, 2 * NW], F32, tag="s", bufs=2, name=f"s{i}")
                s_tiles[i] = st
                # head a: PE rows 0-63; head b: rows 64-127 (row tiling)
                _mm(
                    nc, st[:, 0:NW],
                    k_sb[hp][0:64, ms], q_sb[hp][0:64, gjs],
                    start=True, stop=True, tile_position=(0, 0),
                )
                _mm(
                    nc, st[:, NW : 2 * NW],
                    k_sb[hp][64:128, ms], q_sb[hp][64:128, gjs],
                    start=True, stop=True, tile_position=(64, 0),
                )

            def normalize(hp, qn, last=False):
                # One fast PSUM->SBUF copy frees x_ps for the next block's
                # first PV; the recip/broadcast/mul chain then runs off the
                # PE-critical path from the SBUF staging copy.  The final
                # block skips the staging copy (nothing waits on x_ps).
                gjs = slice(qn * NW, (qn + 1) * NW)
                for i, hnm in enumerate("ab"):
                    xp = x_ps[(hp, qn)][hnm]
                    if not last:
                        xf = sm.tile(
                            [HD + 1, NW], F32, tag=f"xf{hnm}", bufs=2,
                            name=f"xf{hnm}{hp}{qn}",
                        )
                        nc.vector.tensor_copy(xf, xp)
                        if debug and hp == 0 and qn == 0 and hnm == "a":
                            nc.sync.dma_start(dbg_xf, xf)
                    else:
                        xf = xp
                    den = sm.tile([1, NW], F32, tag="den", name=f"dn{hnm}{hp}{qn}")
                    nc.vector.tensor_copy(den, xp[HD : HD + 1, :])
                    recip = sm.tile([1, NW], F32, tag="rec", name=f"rc{hnm}{hp}{qn}")
                    nc.vector.reciprocal_approx_fast(out=recip, in_=den)
                    bc = sm.tile([64, NW], F32, tag="bc", bufs=2, name=f"bc{hnm}{hp}{qn}")
                    nc.gpsimd.partition_broadcast(bc, recip)
                    rows = slice(i * 64, i * 64 + 64)
                    nc.vector.tensor_mul(x_sb[hp][rows, gjs], xf[0:HD, :], bc)

            # ---- prelude: only the minimum before the first qk pair; the
            # rest is emitted right after qk(0)/qk(1) and overlaps the first
            # attention steps.  Everything runs on the "s" psum tag with DVE
            # copies so a chained rep's prelude overlaps the previous rep's
            # tail (ACT queue stays exp-only, "mg" stays with the old tail).

            # ---- hooks keyed by global step --------------------------------
            from collections import defaultdict
            from functools import partial

            hooks = defaultdict(list)
            # vt_proj 7..15 (PV(mt) is emitted at step mt+2)
            for mt in range(7, MT):
                hooks[mt - 3].append(partial(vt_proj, mt))
            # q_proj(qn+1): ct0 before block 2(qn+1) (step 32(qn+1)-2),
            # ct1 before block 2(qn+1)+1
            for qn in range(NQ - 1):
                hooks[32 * qn + 13].append(partial(q_proj, qn + 1, False, [0]))
                hooks[32 * qn + 21].append(partial(q_proj, qn + 1, False, [1]))
            # merge(qn) interleaved into the blocks of window qn+1
            for qn in range(NQ - 1):
                for ot in range(KT):
                    blk = 2 * (qn + 1) + (ot // 2)
                    hooks[blk * MT + 5 + 6 * (ot % 2)].append(partial(merge_unit, ot, qn))

            pr_tiles = {}

            def is_off(i):
                return steps[i][2] in offload_mts and i >= OFFLOAD_START

            # PV emission is deferred: exact tiles 2 steps after their exp
            # (3 for mt==0 to decouple x_ps reuse from the previous block's
            # staging copy), offloaded tiles 6 steps (DVE latency headroom).
            # The block's last-emitted PV carries stop=True.
            def pv_due(i):
                mt = steps[i][2]
                if i >= NS - MT:  # last block: nothing left to overlap with
                    return i + (4 if is_off(i) else 2 if mt == 0 else 1)
                return i + (6 if is_off(i) else 3 if mt == 0 else 2)

            last_pv = {}  # block index -> step that must carry stop=True
            for i in range(NS):
                last_pv[i // MT] = max(
                    last_pv.get(i // MT, (0, 0)), (pv_due(i), i)
                )
            last_pv = {b: j for b, (_, j) in last_pv.items()}

            def pv(i):
                hp, qn, mt = steps[i]
                if mt == 0:
                    x_ps[(hp, qn)] = {
                        hnm: pp.tile(
                            [HD + 1, NW], F32, tag=f"x{hnm}", bufs=1,
                            name=f"xp{hnm}{hp}{qn}",
                        )
                        for hnm in "ab"
                    }
                pr = pr_tiles.pop(i)
                stop = last_pv[i // MT] == i
                for j, hnm in enumerate("ab"):
                    _mm(
                        nc, x_ps[(hp, qn)][hnm],
                        vt_sb[mt][:, (2 * hp + j) * (HD + 1) : (2 * hp + j + 1) * (HD + 1)],
                        pr[:, j * NW : (j + 1) * NW],
                        start=(mt == 0), stop=stop,
                    )
                if stop:
                    normalize(hp, qn, last=(i // MT == len(border) - 1))

            it_tiles = {}

            def op1(i):
                # Schraudolph exp on DVE, hoisted one step early: builds int
                # bits from PSUM scores as soon as qk(i) lands, so the s-buf
                # frees without gating the qk chain on DVE latency.
                it = sm.tile(
                    [128, 2 * NW], I32, tag=f"it{i % 2}", bufs=1, name=f"it{i}"
                )
                nc.vector.tensor_scalar(
                    it, s_tiles.pop(i), SCHRA_A, SCHRA_B, op0=ALU.mult, op1=ALU.add
                )
                it_tiles[i] = it

            import heapq

            pv_q = []  # (due, step)
            noff = 0

            k_proj(0, on_act=False, ptag="s")
            q_proj(0, on_act=False, ptag="s")
            qk(0)
            qk(1)
            for g in range(1, NQ):
                k_proj(g, on_act=False, ptag="s")
            for mt in range(7):
                vt_proj(mt, ptag="s")
            if is_off(0):
                op1(0)
            for i in range(NS):
                hp, qn, mt = steps[i]
                if i + 1 < NS and is_off(i + 1):
                    op1(i + 1)
                if is_off(i):
                    pr = sm.tile(
                        [128, 2 * NW], F32R, tag=f"po{noff % 3}", bufs=1, name=f"pr{i}"
                    )
                    noff += 1
                    nc.vector.tensor_copy(pr, it_tiles.pop(i).bitcast(F32))
                else:
                    pr = sm.tile(
                        [128, 2 * NW], F32R, tag=f"pr{i % 4}", bufs=1, name=f"pr{i}"
                    )
                    nc.scalar.activation(pr, s_tiles.pop(i), EXP, scale=0.125)
                pr_tiles[i] = pr
                if debug and i == 0:
                    nc.sync.dma_start(dbg_pr, pr.bitcast(F32))
                if i + 2 < NS:
                    qk(i + 2)
                heapq.heappush(pv_q, (pv_due(i), i))
                while pv_q and pv_q[0][0] <= i:
                    pv(heapq.heappop(pv_q)[1])
                for fn in hooks.get(i, ()):
                    fn()
            while pv_q:
                pv(heapq.heappop(pv_q)[1])
            # tail: last window's merges
            for ot in range(KT):
                merge_unit(ot, NQ - 1, on_act=(ot % 2 == 0))
            if debug:
                nc.sync.dma_start(dbg_q, q_sb[0].bitcast(F32))
                nc.sync.dma_start(dbg_k, k_sb[0].bitcast(F32))
                nc.sync.dma_start(dbg_vt, vt_sb[0].bitcast(F32))
                nc.sync.dma_start(dbg_x, x_sb[0].bitcast(F32))

        for rep in range(reps):
            emit_body(rep)

    nc.compile()
    return nc


def make_in_maps(query, key, value, Wq, bq, Wk, Wv, Wm, n_cores=NCORES):
    import ml_dtypes

    bf16 = ml_dtypes.bfloat16
    query = np.asarray(query, np.float32)
    key = np.asarray(key, np.float32)
    value = np.asarray(value, np.float32)
    Wq = np.asarray(Wq, np.float32)
    bq = np.asarray(bq, np.float32)
    Wk = np.asarray(Wk, np.float32)
    Wv = np.asarray(Wv, np.float32)
    Wm = np.asarray(Wm, np.float32)
    in_maps = []
    for c in range(n_cores):
        b, hg = c // 2, c % 2
        heads = [hg * HPC + i for i in range(HPC)]
        mych = np.array([d * H + h for h in heads for d in range(HD)])
        in_maps.append(
            {
                "xq": np.ascontiguousarray(query[b].astype(bf16)),
                "xk": np.ascontiguousarray(key[b].astype(bf16)),
                "xv": np.ascontiguousarray(value[b].astype(bf16)),
                "wqT": np.ascontiguousarray(Wq[mych].T.astype(bf16)),
                "wkT": np.ascontiguousarray(Wk[mych].T.astype(bf16)),
                "wvT": np.ascontiguousarray(Wv[mych].T.astype(bf16)),
                "wmT": np.ascontiguousarray(Wm[:, mych].T),
                "bq2": np.ascontiguousarray(bq[mych].reshape(CT, 128).T),
                "onesc": np.ones((128, HPC), np.float32),
            }
        )
    return in_maps


def prep_inputs(inputs):
    """dev_hwtime.py hook: full-input dict -> per-core in_maps."""
    return make_in_maps(
        inputs["query"], inputs["key"], inputs["value"], inputs["Wq"],
        inputs["bq"], inputs["Wk"], inputs["Wv"], inputs["Wm"],
    )


_PROG = {}


def _get_program():
    if "p" not in _PROG:
        _PROG["p"] = build_program()
    return _PROG["p"]


def kernel(query, key, value, Wq, bq, Wk, bk, Wv, bv, Wm, bm):
    nc = _get_program()
    in_maps = make_in_maps(query, key, value, Wq, bq, Wk, Wv, Wm)
    res = run_bass_kernel_spmd(nc, in_maps, list(range(NCORES))).results
    bm_eff = (np.asarray(Wm, np.float64) @ np.asarray(bv, np.float64)).astype(
        np.float32
    ) + np.asarray(bm, np.float32)
    out = np.empty((B, D, N), np.float32)
    for b in range(B):
        out[b] = res[2 * b]["out"] + res[2 * b + 1]["out"] + bm_eff[:, None]
    return out
